# revision 5
# baseline (speedup 1.0000x reference)
"""Multi-plane hashgrid encoding + MLP for Trainium2 (Bass), 8-core data-parallel.

Strategy: shard the N=1M points across 8 NeuronCores (131072 each); replicate
the 6x16 hash tables and MLP weights. Per 128-point chunk (hardware For_i
loop), compute all 96 (plane,level) corner hashes with integer DVE ops,
fetch the 4 bilinear corners per (plane,level) with [128,1]-shaped indirect
row-gathers (one row per partition - the only indirect-DMA shape that is
correct on this toolchain), blend on the vector engine, and run the 3-layer
MLP on the tensor engine (PE transposes + PSUM-accumulated matmuls).
"""

import sys

for p in ("/opt/trn_rl_repo", "/root/.axon_site", "/root/.axon_site/_ro/trn_rl_repo",
          "/root/.axon_site/_ro/pypackages", "/opt/pypackages"):
    if p not in sys.path:
        sys.path.append(p)

import numpy as np

import concourse.bass as bass
import concourse.mybir as mybir
import concourse.tile as tile
from concourse import bacc
from concourse.bass import ds
from concourse.bass_utils import run_bass_kernel_spmd
from concourse.masks import make_identity

dt = mybir.dt
Alu = mybir.AluOpType

N = 1048576
NCORES = 8
NC_PTS = N // NCORES          # 131072
L = 16
T = 524288                    # 2**19
F = 2
PLANES = 6
NPL = PLANES * L              # 96
BASE = 16.0
GROWTH = 1.3819
RES = np.asarray(BASE * GROWTH ** np.arange(L), dtype=np.float32)
# PRIME1 mod 2**19 = 489905 = 478*1024 + 433 (all products stay < 2**21)
C_A, C_B, C_FULL = 433, 478, 489905
MASK19 = 0x7FFFF
P = 128

_nc_cache = {}


def _build(n_pts):
    nc = bacc.Bacc("TRN2", target_bir_lowering=False, debug=False)

    u_d = nc.dram_tensor("u", [n_pts, PLANES], dt.float32, kind="ExternalInput")
    v_d = nc.dram_tensor("v", [n_pts, PLANES], dt.float32, kind="ExternalInput")
    tab_ds = [nc.dram_tensor(f"tab{i}", [L * T, F], dt.float32, kind="ExternalInput")
              for i in range(PLANES)]
    res_d = nc.dram_tensor("res", [P, NPL], dt.float32, kind="ExternalInput")
    plt_d = nc.dram_tensor("plt", [P, NPL], dt.int32, kind="ExternalInput")
    w1_d = nc.dram_tensor("w1p", [204, 64], dt.float32, kind="ExternalInput")
    w2_d = nc.dram_tensor("w2", [64, 64], dt.float32, kind="ExternalInput")
    w3_d = nc.dram_tensor("w3", [64, 3], dt.float32, kind="ExternalInput")
    out_d = nc.dram_tensor("out", [n_pts, 3], dt.float32, kind="ExternalOutput")

    with tile.TileContext(nc) as tc:
        with (
            tc.tile_pool(name="cst", bufs=1) as cst,
            tc.tile_pool(name="sb", bufs=1) as sb,
            tc.tile_pool(name="ps", bufs=1, space="PSUM") as ps,
        ):
            # ---- static constants in SBUF ----
            res_t = cst.tile([P, NPL], dt.float32, tag="res_t")
            nc.sync.dma_start(res_t[:], res_d[:])
            plt_t = cst.tile([P, NPL], dt.int32, tag="plt_t")
            nc.sync.dma_start(plt_t[:], plt_d[:])
            w1a = cst.tile([P, 64], dt.float32, tag="w1a")
            nc.sync.dma_start(w1a[:], w1_d[0:128, :])
            w1b = cst.tile([76, 64], dt.float32, tag="w1b")
            nc.sync.dma_start(w1b[:], w1_d[128:204, :])
            w2_t = cst.tile([64, 64], dt.float32, tag="w2_t")
            nc.sync.dma_start(w2_t[:], w2_d[:])
            w3_t = cst.tile([64, 3], dt.float32, tag="w3_t")
            nc.sync.dma_start(w3_t[:], w3_d[:])
            ident = cst.tile([P, P], dt.float32, tag="ident")
            make_identity(nc, ident[:])

            def floor_int(x_f32, tag):
                """floor of non-negative f32 -> (int32 tile, f32 float(floor))."""
                xi = sb.tile([P, NPL], dt.int32, tag=tag + "_i")
                nc.vector.tensor_copy(xi[:], x_f32[:])          # round-to-nearest
                xf = sb.tile([P, NPL], dt.float32, tag=tag + "_f")
                nc.vector.tensor_copy(xf[:], xi[:])
                d = sb.tile([P, NPL], dt.int32, tag=tag + "_d")
                nc.vector.tensor_tensor(d[:], xf[:], x_f32[:], op=Alu.is_gt)
                nc.vector.tensor_tensor(xi[:], xi[:], d[:], op=Alu.subtract)
                nc.vector.tensor_copy(xf[:], xi[:])
                return xi, xf

            with tc.For_i(0, n_pts, P, hint_engines=(mybir.EngineType.Pool,)) as ib:
                u6 = sb.tile([P, PLANES], dt.float32, tag="u6")
                nc.sync.dma_start(u6[:], u_d[ds(ib, P), :])
                v6 = sb.tile([P, PLANES], dt.float32, tag="v6")
                nc.sync.dma_start(v6[:], v_d[ds(ib, P), :])

                u96 = sb.tile([P, NPL], dt.float32, tag="u96")
                v96 = sb.tile([P, NPL], dt.float32, tag="v96")
                for p in range(PLANES):
                    nc.vector.tensor_copy(
                        u96[:, p * L:(p + 1) * L],
                        u6[:, p:p + 1].to_broadcast([P, L]),
                    )
                    nc.vector.tensor_copy(
                        v96[:, p * L:(p + 1) * L],
                        v6[:, p:p + 1].to_broadcast([P, L]),
                    )

                posu = sb.tile([P, NPL], dt.float32, tag="posu")
                nc.vector.tensor_tensor(posu[:], u96[:], res_t[:], op=Alu.mult)
                posv = sb.tile([P, NPL], dt.float32, tag="posv")
                nc.vector.tensor_tensor(posv[:], v96[:], res_t[:], op=Alu.mult)

                xi, xf = floor_int(posu, "x")
                yi, yf = floor_int(posv, "y")
                wx = sb.tile([P, NPL], dt.float32, tag="wx")
                nc.vector.tensor_tensor(wx[:], posu[:], xf[:], op=Alu.subtract)
                wy = sb.tile([P, NPL], dt.float32, tag="wy")
                nc.vector.tensor_tensor(wy[:], posv[:], yf[:], op=Alu.subtract)

                # hash of y row:  g0 = (yi*489905) mod 2**19, g1 = same for yi+1
                ha = sb.tile([P, NPL], dt.int32, tag="ha")
                nc.vector.tensor_scalar(ha[:], yi[:], C_A, None, op0=Alu.mult)
                hb = sb.tile([P, NPL], dt.int32, tag="hb")
                nc.vector.tensor_scalar(hb[:], yi[:], C_B, None, op0=Alu.mult)
                nc.vector.tensor_scalar(hb[:], hb[:], 511, 10,
                                        op0=Alu.bitwise_and,
                                        op1=Alu.logical_shift_left)
                g0 = sb.tile([P, NPL], dt.int32, tag="g0")
                nc.vector.tensor_tensor(g0[:], ha[:], hb[:], op=Alu.add)
                nc.vector.tensor_scalar(g0[:], g0[:], MASK19, None,
                                        op0=Alu.bitwise_and)
                g1 = sb.tile([P, NPL], dt.int32, tag="g1")
                nc.vector.tensor_scalar(g1[:], g0[:], C_FULL, None, op0=Alu.add)
                nc.vector.tensor_scalar(g1[:], g1[:], MASK19, None,
                                        op0=Alu.bitwise_and)

                xi1 = sb.tile([P, NPL], dt.int32, tag="xi1")
                nc.vector.tensor_scalar(xi1[:], xi[:], 1, None, op0=Alu.add)

                def offsets(xc, gc, tag):
                    o = sb.tile([P, NPL], dt.int32, tag=tag)
                    nc.vector.tensor_tensor(o[:], xc[:], gc[:], op=Alu.bitwise_xor)
                    nc.vector.tensor_tensor(o[:], o[:], plt_t[:], op=Alu.add)
                    return o

                o00 = offsets(xi, g0, "o00")
                o10 = offsets(xi1, g0, "o10")
                o01 = offsets(xi, g1, "o01")
                o11 = offsets(xi1, g1, "o11")

                gt = {}
                for cname, off in (("00", o00), ("10", o10), ("01", o01), ("11", o11)):
                    g = sb.tile([P, NPL * F], dt.float32, tag="gt" + cname)
                    gt[cname] = g
                    for pl in range(NPL):
                        nc.gpsimd.indirect_dma_start(
                            out=g[:, pl * F:(pl + 1) * F],
                            out_offset=None,
                            in_=tab_ds[pl // L][:],
                            in_offset=bass.IndirectOffsetOnAxis(
                                ap=off[:, pl:pl + 1], axis=0),
                        )

                # duplicate weights per feature: [P, NPL] -> [P, NPL*F]
                wx2 = sb.tile([P, NPL, F], dt.float32, tag="wx2")
                nc.vector.tensor_copy(wx2[:], wx[:, :, None].to_broadcast([P, NPL, F]))
                wy2 = sb.tile([P, NPL, F], dt.float32, tag="wy2")
                nc.vector.tensor_copy(wy2[:], wy[:, :, None].to_broadcast([P, NPL, F]))
                wx2v = wx2[:].rearrange("p a b -> p (a b)")
                wy2v = wy2[:].rearrange("p a b -> p (a b)")

                enc = sb.tile([P, 204], dt.float32, tag="enc")

                # bilinear blend: t0 = c00 + wx*(c10-c00); t1 = c01 + wx*(c11-c01)
                #                 enc = t0 + wy*(t1-t0)
                t0 = sb.tile([P, NPL * F], dt.float32, tag="t0")
                nc.vector.tensor_tensor(t0[:], gt["10"][:], gt["00"][:], op=Alu.subtract)
                nc.vector.tensor_tensor(t0[:], t0[:], wx2v, op=Alu.mult)
                nc.vector.tensor_tensor(t0[:], t0[:], gt["00"][:], op=Alu.add)
                t1 = sb.tile([P, NPL * F], dt.float32, tag="t1")
                nc.vector.tensor_tensor(t1[:], gt["11"][:], gt["01"][:], op=Alu.subtract)
                nc.vector.tensor_tensor(t1[:], t1[:], wx2v, op=Alu.mult)
                nc.vector.tensor_tensor(t1[:], t1[:], gt["01"][:], op=Alu.add)
                nc.vector.tensor_tensor(t1[:], t1[:], t0[:], op=Alu.subtract)
                nc.vector.tensor_tensor(t1[:], t1[:], wy2v, op=Alu.mult)
                nc.vector.tensor_tensor(enc[:, 0:NPL * F], t1[:], t0[:], op=Alu.add)

                nc.vector.tensor_copy(enc[:, 192:198], u6[:])
                nc.vector.tensor_copy(enc[:, 198:204], v6[:])

                # ---- MLP ----
                etా = None
                encta_p = ps.tile([P, P], dt.float32, tag="encta_p")
                nc.tensor.transpose(encta_p[:], enc[:, 0:128], ident[:])
                encta = sb.tile([P, P], dt.float32, tag="encta")
                nc.vector.tensor_copy(encta[:], encta_p[:])
                enctb_p = ps.tile([76, P], dt.float32, tag="enctb_p")
                nc.tensor.transpose(enctb_p[:], enc[:, 128:204], ident[:])
                enctb = sb.tile([76, P], dt.float32, tag="enctb")
                nc.vector.tensor_copy(enctb[:], enctb_p[:])

                h1p = ps.tile([P, 64], dt.float32, tag="h1p")
                nc.tensor.matmul(h1p[:], lhsT=encta[:], rhs=w1a[:], start=True, stop=False)
                nc.tensor.matmul(h1p[:], lhsT=enctb[:], rhs=w1b[:], start=False, stop=True)
                h1 = sb.tile([P, 64], dt.float32, tag="h1")
                nc.scalar.activation(h1[:], h1p[:], mybir.ActivationFunctionType.Relu)

                h1tp = ps.tile([64, P], dt.float32, tag="h1tp")
                nc.tensor.transpose(h1tp[:], h1[:], ident[:])
                h1t = sb.tile([64, P], dt.float32, tag="h1t")
                nc.vector.tensor_copy(h1t[:], h1tp[:])
                h2p = ps.tile([P, 64], dt.float32, tag="h2p")
                nc.tensor.matmul(h2p[:], lhsT=h1t[:], rhs=w2_t[:], start=True, stop=True)
                h2 = sb.tile([P, 64], dt.float32, tag="h2")
                nc.scalar.activation(h2[:], h2p[:], mybir.ActivationFunctionType.Relu)

                h2tp = ps.tile([64, P], dt.float32, tag="h2tp")
                nc.tensor.transpose(h2tp[:], h2[:], ident[:])
                h2t = sb.tile([64, P], dt.float32, tag="h2t")
                nc.vector.tensor_copy(h2t[:], h2tp[:])
                o3p = ps.tile([P, 3], dt.float32, tag="o3p")
                nc.tensor.matmul(o3p[:], lhsT=h2t[:], rhs=w3_t[:], start=True, stop=True)
                o3 = sb.tile([P, 3], dt.float32, tag="o3")
                nc.vector.tensor_copy(o3[:], o3p[:])
                nc.sync.dma_start(out_d[ds(ib, P), :], o3[:])

    nc.compile()
    return nc


def _host_prep(inputs, n_pts_core):
    """Build the per-core input maps (pure layout work)."""
    pts = [inputs["points_xy"], inputs["points_xz"], inputs["points_yz"],
           inputs["points_xt"], inputs["points_yt"], inputs["points_zt"]]
    tables = inputs["tables"]
    U = np.stack([p[:, 0] for p in pts], axis=1).astype(np.float32)  # [N, 6]
    V = np.stack([p[:, 1] for p in pts], axis=1).astype(np.float32)

    tab_planes = [np.ascontiguousarray(tables[i].reshape(L * T, F)).astype(np.float32)
                  for i in range(PLANES)]

    res_col = np.tile(RES, PLANES)                       # [96] per (plane,level)
    res_t = np.broadcast_to(res_col[None, :], (P, NPL)).copy()
    plt_col = ((np.arange(NPL, dtype=np.int64) % L) * T).astype(np.int32)
    plt_t = np.broadcast_to(plt_col[None, :], (P, NPL)).copy()

    # permute W1 rows to match our enc column order
    perm = np.zeros(204, np.int64)
    for pl in range(NPL):
        plane, lev = pl // L, pl % L
        for f in range(F):
            perm[2 * pl + f] = plane * 34 + lev * 2 + f
    for plane in range(PLANES):
        perm[192 + plane] = plane * 34 + 32
        perm[198 + plane] = plane * 34 + 33
    w1p = np.ascontiguousarray(inputs["W1"][perm, :]).astype(np.float32)

    maps = []
    for c in range(NCORES):
        s = slice(c * n_pts_core, (c + 1) * n_pts_core)
        maps.append({
            "u": np.ascontiguousarray(U[s]),
            "v": np.ascontiguousarray(V[s]),
            **{f"tab{i}": tab_planes[i] for i in range(PLANES)},
            "res": res_t,
            "plt": plt_t,
            "w1p": w1p,
            "w2": np.ascontiguousarray(inputs["W2"]).astype(np.float32),
            "w3": np.ascontiguousarray(inputs["W3"]).astype(np.float32),
        })
    return maps


def kernel(**inputs):
    n_pts_core = inputs["points_xy"].shape[0] // NCORES
    if n_pts_core not in _nc_cache:
        _nc_cache[n_pts_core] = _build(n_pts_core)
    nc = _nc_cache[n_pts_core]
    maps = _host_prep(inputs, n_pts_core)
    res = run_bass_kernel_spmd(nc, maps, core_ids=list(range(NCORES)))
    out = np.concatenate([np.asarray(r["out"]) for r in res.results], axis=0)
    return out.astype(np.float32)


if __name__ == "__main__":
    rng = np.random.default_rng(0)
    n = int(sys.argv[1]) if len(sys.argv) > 1 else 2048 * NCORES
    inputs = {k: rng.random((n, 2), dtype=np.float32) for k in
              ["points_xy", "points_xz", "points_yz", "points_xt", "points_yt", "points_zt"]}
    inputs["tables"] = (rng.random((PLANES, L, T, F), dtype=np.float32) * 2e-4 - 1e-4).astype(np.float32)
    inputs["W1"] = rng.standard_normal((204, 64), dtype=np.float32)
    inputs["W2"] = rng.standard_normal((64, 64), dtype=np.float32)
    inputs["W3"] = rng.standard_normal((64, 3), dtype=np.float32)
    out = kernel(**inputs)

    # numpy reference
    def ref_np(inputs):
        pts = [inputs["points_xy"], inputs["points_xz"], inputs["points_yz"],
               inputs["points_xt"], inputs["points_yt"], inputs["points_zt"]]
        parts = []
        for i in range(6):
            pn = pts[i]
            feats = []
            for lev in range(L):
                pos = pn * RES[lev]
                pf = np.floor(pos)
                w = pos - pf
                pi = pf.astype(np.int64)
                def corner(dx, dy):
                    cx = (pi[:, 0] + dx).astype(np.uint32)
                    cy = (pi[:, 1] + dy).astype(np.uint32)
                    h = (cx * np.uint32(1)) ^ (cy * np.uint32(2654435761))
                    return inputs["tables"][i, lev][(h % np.uint32(T)).astype(np.int64)]
                wx, wy = w[:, 0:1], w[:, 1:2]
                feats.append(corner(0, 0) * (1 - wx) * (1 - wy)
                             + corner(1, 0) * wx * (1 - wy)
                             + corner(0, 1) * (1 - wx) * wy
                             + corner(1, 1) * wx * wy)
            parts.append(np.concatenate(feats, axis=1))
            parts.append(pn)
        enc = np.concatenate(parts, axis=1).astype(np.float32)
        h = np.maximum(enc @ inputs["W1"], 0)
        h = np.maximum(h @ inputs["W2"], 0)
        return h @ inputs["W3"]

    exp = ref_np(inputs)
    err = np.abs(out - exp).max() / (np.abs(exp).max() + 1e-30)
    print("out", out.shape, "relerr", err)
